# revision 6
# baseline (speedup 1.0000x reference)
"""Trainium2 Bass kernel for nn_CPAMDec_Mix (dual cross-attention decoder block).

Math per batch sample b (C=512, C4=128, K=64, N=W*H=4096):
    pv1 = wv @ y1^T + bv          [C, K]   (host-precomputed, scale folded)
    pv2 = wv @ y2^T + bv          [C, K]   (host-precomputed, scale1 folded)
    q^T = wq @ x2 + bq            [C4, N]
    kk  = y2 @ wk^T + bk          [K, C4]  (host-precomputed)
    energy = q @ kk^T             [N, K]
    att = softmax(|energy|, -1)   [N, K]
    out1 = scale  * pv1 @ att^T + x1
    out2 = scale1 * pv2 @ att^T + x2

Sharding: pure data parallel — sample b on core b (B == n_cores == 8).

v3 scheduling notes (from perfetto traces of v1 @ 87.6us / v2 @ 76.5us):
 - ~6.5us of NEFF preamble (sem barriers + TENSOR_LOAD) is fixed cost.
 - The kernel is ring-bound: ~16.3 MiB of HBM traffic ~= 47.5us at 358
   GB/s.  Everything else must hide under that.
 - v2 was PE-cadence-bound in the out phase (~11us/quarter vs 5.9us of
   store drain).  v3 removes PE work: pv/kk are host-precomputed (tiny
   [C,K]/[K,C4] projections, 0.4% of FLOPs), and the out2 residual moves
   from PE identity-matmuls to the (idle) GPSIMD engine operating on the
   ACT-evacuated SBUF tile.
 - Engine balance per quarter: PE ~6.7us (po 16 + qproj 8 + energy 8 +
   transpose 8 matmuls), DVE ~6.1 (o1 epilogues + softmax), ACT ~6.5
   (qT, |e|, exp, po2 evacs), GPSIMD ~4.5 (o2 residual adds).
 - All loads flat [128, 4096] quarters; ONE [128, 1730] blob carries
   every weight/constant.  Stores flat per HALF-quarter so the tail
   after the last compute is only ~0.5 MiB deep.
Two attention chains stay in flight as in v1/v2.
"""

import numpy as np
import ml_dtypes

import concourse.bass as bass
import concourse.mybir as mybir
import concourse.tile as tile
from concourse import bacc
from concourse.bass_utils import run_bass_kernel_spmd

F32 = mybir.dt.float32
BF16 = mybir.dt.bfloat16
U32 = mybir.dt.uint32
NP_BF16 = np.dtype(ml_dtypes.bfloat16)
AX = mybir.AxisListType
OP = mybir.AluOpType
AF = mybir.ActivationFunctionType

B, C, W, H, K = 8, 512, 64, 64, 64
C4 = C // 4
N = W * H            # 4096
NT = 512             # columns per f32 psum bank / matmul
NQ = 1024            # quarter width
CC = C // 128        # 4 chunks of 128 over the channel dim
NHALF = NQ // NT     # 2
NQuarters = N // NQ

# blob column offsets (bf16 columns; bq is f32 bitcast at offset 0)
OF_BQ = 0              # [128, 1] f32  == 2 bf16 cols
OF_ID = 2              # [128, 128] identity
OF_WQ = OF_ID + 128    # [128, CC*C4]
OF_KK = OF_WQ + CC * C4   # [128, K] kk^T
OF_PV = OF_KK + K      # [0:64, 512] pv1T | [0:64, 512] pv2T (rows 64-127 unused)
BLOB_W = OF_PV + 2 * C   # 1730

_CACHE = {}


class _AttQuarter:
    """Attention for one quarter, split into 4 emission stages so two
    chains can be interleaved with the out-phase."""

    def __init__(self, nc, x2q, ctx):
        self.nc = nc
        self.x2q = x2q
        self.ctx = ctx

    def stage0(self):  # q-projection into one 2-bank psum tile + qT act
        nc, c = self.nc, self.ctx
        psum_q = c["psq"].tile([C4, NQ], F32, tag="psq")
        for half in range(NHALF):
            o = half * NT
            for cc in range(CC):
                nc.tensor.matmul(
                    psum_q[:, o : o + NT],
                    lhsT=c["wqT"][:, cc * C4 : (cc + 1) * C4],
                    rhs=self.x2q[:, cc * NQ + o : cc * NQ + o + NT],
                    start=(cc == 0),
                    stop=(cc == CC - 1),
                )
        self.qT = c["qpool"].tile([C4, NQ], BF16, tag="qT")
        nc.scalar.activation(self.qT[:], psum_q[:], AF.Identity, bias=c["bq"])

    def stage1(self):  # energy + |e| (ACT) + exp (ACT)
        nc, c = self.nc, self.ctx
        self.psum_e = c["ept"].tile([128, 8 * K], F32, tag="ept")
        for s in range(8):
            nc.tensor.matmul(
                self.psum_e[:, s * K : (s + 1) * K],
                lhsT=self.qT[:, s * 128 : (s + 1) * 128],
                rhs=c["kkT"],
                start=True,
                stop=True,
            )
        self.eabs = c["spool"].tile([128, 8 * K], F32, tag="eabs")
        nc.scalar.activation(self.eabs[:], self.psum_e[:], AF.Abs)
        self.eexp = c["spool"].tile([128, 8 * K], BF16, tag="eexp")
        nc.scalar.activation(self.eexp[:], self.eabs[:], AF.Exp)

    def stage2(self):  # softmax normalize + transpose
        nc, c = self.nc, self.ctx
        rsum = c["spool"].tile([128, 8], F32, tag="rsum")
        nc.vector.tensor_reduce(
            rsum[:],
            self.eexp[:].rearrange("p (g d) -> p g d", g=8),
            axis=AX.X,
            op=OP.add,
        )
        rrec = c["spool"].tile([128, 8], F32, tag="rrec")
        nc.vector.reciprocal(rrec[:], rsum[:])
        att = c["spool"].tile([128, 8 * K], BF16, tag="att")
        nc.vector.tensor_tensor(
            att[:].rearrange("p (g d) -> p g d", g=8),
            self.eexp[:].rearrange("p (g d) -> p g d", g=8),
            rrec[:].unsqueeze(2).broadcast_to((128, 8, K)),
            op=OP.mult,
        )
        self.psum_t = c["ept"].tile([K, NQ], BF16, tag="ept")
        for s in range(8):
            nc.tensor.transpose(
                self.psum_t[:, s * 128 : (s + 1) * 128],
                att[:, s * K : (s + 1) * K],
                c["ident"],
            )

    def stage3(self):  # attT -> SBUF
        nc, c = self.nc, self.ctx
        self.aT = c["apool"].tile([K, NQ], BF16, tag="attT")
        nc.vector.tensor_copy(
            self.aT[:].bitcast(U32), self.psum_t[:].bitcast(U32)
        )
        return self.aT


def _build_nc():
    nc = bacc.Bacc("TRN2", target_bir_lowering=False, debug=False)

    # x1/x2 arrive host-rearranged to the SBUF tile layout:
    # xr[p, q*4096 + cc*1024 + n] = x[cc*128 + p, q*1024 + n]
    # so each quarter's load is one flat [128, 4096] slice.  out1/out2 use
    # the SAME flat layout (host un-rearranges after the run).
    x1_d = nc.dram_tensor("x1", [128, N * CC], BF16, kind="ExternalInput")
    x2_d = nc.dram_tensor("x2", [128, N * CC], BF16, kind="ExternalInput")
    blob_d = nc.dram_tensor("blob", [128, BLOB_W], BF16, kind="ExternalInput")
    out1_d = nc.dram_tensor("out1", [128, N * CC], BF16, kind="ExternalOutput")
    out2_d = nc.dram_tensor("out2", [128, N * CC], BF16, kind="ExternalOutput")

    with tile.TileContext(nc) as tc:
        with (
            tc.tile_pool(name="const", bufs=1) as const,
            tc.tile_pool(name="qpool", bufs=2) as qpool,
            tc.tile_pool(name="spool", bufs=2) as spool,
            tc.tile_pool(name="apool", bufs=2) as apool,
            tc.tile_pool(name="o1pool", bufs=2) as o1pool,
            tc.tile_pool(name="o2pool", bufs=2) as o2pool,
            tc.tile_pool(name="epool", bufs=4) as epool,
            tc.tile_pool(name="psq", bufs=1, space="PSUM") as psq,
            tc.tile_pool(name="ept", bufs=2, space="PSUM") as ept,
            tc.tile_pool(name="pso", bufs=4, space="PSUM") as pso,
        ):
            # ---- one flat blob DMA brings every weight/constant ----
            blob = const.tile([128, BLOB_W], BF16)
            nc.sync.dma_start(out=blob[:], in_=blob_d[:])

            bq_sb = blob[:, OF_BQ : OF_BQ + 2].bitcast(F32)
            ident = blob[:, OF_ID : OF_ID + 128]
            wqT_sb = blob[:, OF_WQ : OF_WQ + CC * C4]
            kkT_sb = blob[:, OF_KK : OF_KK + K]
            pv1T_sb = blob[0:K, OF_PV : OF_PV + C]
            pv2T_sb = blob[0:K, OF_PV + C : OF_PV + 2 * C]

            # ---- all quarter loads queued upfront in deadline order ----
            x1_sb = [None] * NQuarters
            x2_sb = [None] * NQuarters

            def _load_quarter(dram, q, tag):
                t = const.tile([128, CC * NQ], BF16, tag=tag)
                nc.sync.dma_start(
                    out=t[:], in_=dram[:, q * CC * NQ : (q + 1) * CC * NQ]
                )
                return t

            x2_sb[0] = _load_quarter(x2_d, 0, "x2_0")
            x1_sb[0] = _load_quarter(x1_d, 0, "x1_0")
            x2_sb[1] = _load_quarter(x2_d, 1, "x2_1")
            x2_sb[2] = _load_quarter(x2_d, 2, "x2_2")
            x1_sb[1] = _load_quarter(x1_d, 1, "x1_1")
            x2_sb[3] = _load_quarter(x2_d, 3, "x2_3")
            x1_sb[2] = _load_quarter(x1_d, 2, "x1_2")
            x1_sb[3] = _load_quarter(x1_d, 3, "x1_3")

            ctx = {
                "psq": psq, "ept": ept, "qpool": qpool, "spool": spool,
                "apool": apool, "wqT": wqT_sb, "kkT": kkT_sb,
                "bq": bq_sb, "ident": ident,
            }

            # attention chain 0 runs contiguously at startup, chain 1
            # front half right behind it
            atts = [_AttQuarter(nc, x2_sb[j], ctx) for j in range(NQuarters)]
            a0 = atts[0]
            a0.stage0(); a0.stage1(); a0.stage2()
            aT = a0.stage3()
            atts[1].stage0()
            atts[1].stage1()

            # ---- quarters: out(q) woven with att(q+1) tail + att(q+2) head ----
            for q in range(NQuarters):
                o1 = o1pool.tile([128, CC * NQ], BF16, tag="o1")
                o2 = o2pool.tile([128, CC * NQ], BF16, tag="o2")
                for cc in range(CC):
                    pv1c = pv1T_sb[:, cc * 128 : (cc + 1) * 128]
                    pv2c = pv2T_sb[:, cc * 128 : (cc + 1) * 128]
                    # NT-granular psum tiles (1 bank each, 4-slot pool) so
                    # the PE runs ahead of the DVE/ACT evacuations
                    for i in range(NHALF):
                        ns = slice(cc * NQ + i * NT, cc * NQ + (i + 1) * NT)
                        nt = slice(i * NT, (i + 1) * NT)
                        po1 = pso.tile([128, NT], F32, tag="po")
                        nc.tensor.matmul(
                            po1[:], lhsT=pv1c, rhs=aT[:, nt],
                            start=True, stop=True,
                        )
                        po2 = pso.tile([128, NT], F32, tag="po")
                        nc.tensor.matmul(
                            po2[:], lhsT=pv2c, rhs=aT[:, nt],
                            start=True, stop=True,
                        )
                        # o1 residual fused into the DVE evacuation
                        nc.vector.tensor_tensor(
                            o1[:, ns], po1[:], x1_sb[q][:, ns], op=OP.add,
                        )
                        # o2: ACT evacuates, idle GPSIMD adds the residual
                        o2e = epool.tile([128, NT], BF16, tag="o2e")
                        nc.scalar.activation(o2e[:], po2[:], AF.Identity)
                        nc.gpsimd.tensor_tensor(
                            o2[:, ns], o2e[:], x2_sb[q][:, ns], op=OP.add,
                        )
                    if cc == 0 and q + 1 < NQuarters:
                        atts[q + 1].stage2()
                    elif cc == 1 and q + 1 < NQuarters:
                        # store front half while the back half computes
                        nc.sync.dma_start(
                            out=out1_d[:, q * CC * NQ : q * CC * NQ + 2 * NQ],
                            in_=o1[:, 0 : 2 * NQ],
                        )
                        nc.sync.dma_start(
                            out=out2_d[:, q * CC * NQ : q * CC * NQ + 2 * NQ],
                            in_=o2[:, 0 : 2 * NQ],
                        )
                        aT_next = atts[q + 1].stage3()
                    elif cc == 2 and q + 2 < NQuarters:
                        atts[q + 2].stage0()
                    elif cc == 3 and q + 2 < NQuarters:
                        atts[q + 2].stage1()
                if q + 1 >= NQuarters:
                    # last quarter: store the front half as soon as ready
                    nc.sync.dma_start(
                        out=out1_d[:, q * CC * NQ : q * CC * NQ + 2 * NQ],
                        in_=o1[:, 0 : 2 * NQ],
                    )
                    nc.sync.dma_start(
                        out=out2_d[:, q * CC * NQ : q * CC * NQ + 2 * NQ],
                        in_=o2[:, 0 : 2 * NQ],
                    )
                nc.sync.dma_start(
                    out=out1_d[:, q * CC * NQ + 2 * NQ : (q + 1) * CC * NQ],
                    in_=o1[:, 2 * NQ : CC * NQ],
                )
                nc.sync.dma_start(
                    out=out2_d[:, q * CC * NQ + 2 * NQ : (q + 1) * CC * NQ],
                    in_=o2[:, 2 * NQ : CC * NQ],
                )
                if q + 1 < NQuarters:
                    aT = aT_next
    nc.compile()
    return nc


def _get_nc():
    if "nc" not in _CACHE:
        _CACHE["nc"] = _build_nc()
    return _CACHE["nc"]


def _rearr(x):
    # [C, N] -> [128, q*4096 + cc*1024 + n] (SBUF quarter-tile layout)
    return np.ascontiguousarray(
        x.reshape(CC, 128, N // NQ, NQ).transpose(1, 2, 0, 3).reshape(128, N * CC)
    )


def _unrearr(t):
    # inverse of _rearr: [128, N*CC] -> [C, N]
    return t.reshape(128, N // NQ, CC, NQ).transpose(2, 0, 1, 3).reshape(C, N)


def kernel(x1, y1, x2, y2, wq, bq, wk, bk, wv, bv, scale, scale1, **run_kwargs):
    x1 = np.asarray(x1, np.float32).astype(NP_BF16)
    x2 = np.asarray(x2, np.float32).astype(NP_BF16)
    y1 = np.asarray(y1, np.float32)
    y2 = np.asarray(y2, np.float32)
    wq = np.asarray(wq, np.float32)
    wk = np.asarray(wk, np.float32)
    wv = np.asarray(wv, np.float32)
    bv_ = np.asarray(bv, np.float32).reshape(C)
    bk_ = np.asarray(bk, np.float32).reshape(C4)
    sc1 = float(np.asarray(scale).reshape(-1)[0])
    sc2 = float(np.asarray(scale1).reshape(-1)[0])

    def _chunked(m, inner):
        # [CC*128, inner] -> [128, CC*inner]: dst[p, cc*inner+j] = m[cc*128+p, j]
        return m.reshape(CC, 128, inner).transpose(1, 0, 2).reshape(128, CC * inner)

    blob_shared = np.zeros((128, BLOB_W), NP_BF16)
    blob_shared[:, OF_BQ : OF_BQ + 2] = (
        np.ascontiguousarray(np.asarray(bq, np.float32).reshape(C4, 1))
        .view(np.uint16)
        .view(NP_BF16)
    )
    blob_shared[:, OF_ID : OF_ID + 128] = np.eye(128, dtype=np.float32).astype(NP_BF16)
    blob_shared[:, OF_WQ : OF_WQ + CC * C4] = _chunked(wq.T, C4).astype(NP_BF16)

    in_maps = []
    for b in range(B):
        blob = blob_shared.copy()
        # kk^T[d, k] = wk @ y2[b]^T + bk  (f32 on host, stored bf16)
        blob[:, OF_KK : OF_KK + K] = (wk @ y2[b].T + bk_[:, None]).astype(NP_BF16)
        # pv^T[k, c] = scale * (y @ wv^T + bv)
        blob[0:K, OF_PV : OF_PV + C] = (sc1 * (y1[b] @ wv.T + bv_)).astype(NP_BF16)
        blob[0:K, OF_PV + C : OF_PV + 2 * C] = (
            sc2 * (y2[b] @ wv.T + bv_)
        ).astype(NP_BF16)
        in_maps.append(
            {
                "x1": _rearr(x1[b].reshape(C, N)),
                "x2": _rearr(x2[b].reshape(C, N)),
                "blob": blob,
            }
        )
    nc = _get_nc()
    res = run_bass_kernel_spmd(nc, in_maps, list(range(B)), **run_kwargs)
    _CACHE["last_results"] = res
    out1 = np.stack(
        [
            _unrearr(res.results[b]["out1"].astype(np.float32)).reshape(C, W, H)
            for b in range(B)
        ]
    )
    out2 = np.stack(
        [
            _unrearr(res.results[b]["out2"].astype(np.float32)).reshape(C, W, H)
            for b in range(B)
        ]
    )
    return (out1, out2)
